# revision 16
# baseline (speedup 1.0000x reference)
"""Trainium2 Bass kernel for multi-head attention (B=4, N=2048, C=1024, H=16).

Sharding: 8 cores = (batch b in 0..3) x (head-group hg in 0..1, 8 heads each).
Each core computes, for its (b, hg):
  - QKV projection for its 8 heads (bf16 matmuls, fp32 PSUM, contraction C=1024)
  - attention S^T = K Q^T per head-pair (row-packed K=64 matmuls),
    exp on ACT (no max-subtraction needed: |S|max ~ 9 << 50 clamp), PV with a
    fused ones-row producing the softmax denominators for free
  - normalization fully on-chip (approx-reciprocal + SBUF->SBUF broadcast DMA)
  - output projection y_part = attnT^T @ w_projT, per n-block, interleaved
    into the last pair's attention as PE filler work
Host sums the two partial y's per batch (proj contracts over all 16 heads).

vs v1: all heavy operands bf16 (halves DMA-in/LDWEIGHTS/SBUF), attnT for all
4 pairs stays resident (no DRAM spill/reload), softmax normalize never
round-trips DRAM, PV is software-pipelined one j-step behind exp so ACT never
waits on PE, and the projection runs per n-block inside pair-3 attention.
"""
import sys, os
sys.path.insert(0, "/opt/trn_rl_repo")
import numpy as np
import ml_dtypes
from contextlib import ExitStack

import concourse.bass as bass
import concourse.bacc as bacc
import concourse.tile as tile
import concourse.mybir as mybir
from concourse.bass_utils import run_bass_kernel_spmd

B, N, C, H, D = 4, 2048, 1024, 16, 64
P = 128
NH = H // 2              # 8 heads per core
CH = NH * D              # 512: per-core channel slice
NPAIR = NH // 2          # 4 head-pairs per core
NBLK = 4                 # nq blocks of 512
BLK = N // NBLK          # 512
NT = N // P              # 16 key tiles
CC = C // P              # 8 contraction chunks
F32 = mybir.dt.float32
F32R = mybir.dt.float32r
BF16 = mybir.dt.bfloat16
AF = mybir.ActivationFunctionType
BF = ml_dtypes.bfloat16


def build_program():
    nc = bacc.Bacc(None, target_bir_lowering=False)
    xT = nc.declare_dram_parameter("xT", [C, N], BF16, isOutput=False)
    wqT = nc.declare_dram_parameter("wqT", [C, CH], BF16, isOutput=False)
    wkT = nc.declare_dram_parameter("wkT", [C, CH], BF16, isOutput=False)
    wvT = nc.declare_dram_parameter("wvT", [C, CH], BF16, isOutput=False)
    bq = nc.declare_dram_parameter("bq", [CH], F32, isOutput=False)
    bk = nc.declare_dram_parameter("bk", [CH], F32, isOutput=False)
    wpT = nc.declare_dram_parameter("wpT", [CH, C], BF16, isOutput=False)
    beff = nc.declare_dram_parameter("beff", [C], F32, isOutput=False)
    ones_in = nc.declare_dram_parameter("ones_in", [P], F32, isOutput=False)
    yT = nc.declare_dram_parameter("yT", [C, N], BF16, isOutput=True)

    with tile.TileContext(nc) as tc, ExitStack() as ctx:
        sb = ctx.enter_context(tc.tile_pool(name="sb", bufs=1))
        ps = ctx.enter_context(tc.tile_pool(name="ps", bufs=1, space="PSUM"))
        dr = ctx.enter_context(tc.tile_pool(name="dr", bufs=1, space="DRAM"))

        # ---- loads, spread over three trigger queues so transfers start in
        # parallel: xT chunks on Sync, q/k weights + biases on Scalar (ACT is
        # idle until the first exp), wvT/wpT on GpSimd.  wq/wk load whole
        # (1KB-contiguous rows, one trigger) and pairs use views into them.
        xT_c = [sb.tile([P, N], BF16, tag="xT", bufs=CC, name=f"xTc{c}") for c in range(CC)]
        wk_all = sb.tile([P, CC, CH], BF16, tag="wk")
        nc.scalar.dma_start(wk_all[:], wkT.rearrange("(cc p) m -> p cc m", p=P))
        for c in range(CC):
            nc.sync.dma_start(xT_c[c][:], xT[c * P:(c + 1) * P, :])
        bk_sb = sb.tile([P, NPAIR], F32, tag="biask")
        nc.scalar.dma_start(bk_sb[:], bk.rearrange("(t p) -> p t", p=P))
        wq_all = sb.tile([P, CC, CH], BF16, tag="wq")
        nc.scalar.dma_start(wq_all[:], wqT.rearrange("(cc p) m -> p cc m", p=P))
        bq_sb = sb.tile([P, NPAIR], F32, tag="biasq")
        nc.scalar.dma_start(bq_sb[:], bq.rearrange("(t p) -> p t", p=P))
        v_sb = sb.tile([P, NT, NH, D + 1], F32R, tag="v")
        ones_col = sb.tile([P, 1], F32, tag="onesc")
        nc.scalar.dma_start(ones_col[:], ones_in.rearrange("(p o) -> p o", o=1))
        nc.vector.tensor_copy(v_sb[:, :, :, D:D + 1], ones_col[:].to_broadcast((P, NT, NH, 1)))

        def wq_view(pair):
            return wq_all[:, :, pair * P:(pair + 1) * P]

        def wk_view(pair):
            return wk_all[:, :, pair * P:(pair + 1) * P]

        qT = [None] * NPAIR
        kT = [None] * NPAIR

        def alloc_qk(pair):
            # per-block tiles: S matmuls for key-tile j / query-block b then only
            # depend on the specific q/k block copybacks, not the whole pair
            qT[pair] = [sb.tile([P, BLK], BF16, tag="qT", bufs=2 * NBLK, name=f"qT{pair}_{b}")
                        for b in range(NBLK)]
            kT[pair] = [sb.tile([P, BLK], BF16, tag="kT", bufs=2 * NBLK, name=f"kT{pair}_{b}")
                        for b in range(NBLK)]

        # ---- V for all heads + q^T/k^T for pair 0: chunk-major waves over 6
        # psum slots, so the first matmuls only wait on xT chunk 0's DMA.
        wvT_sb = sb.tile([P, CC, CH], BF16, tag="wbig")
        nc.gpsimd.dma_start(wvT_sb[:], wvT.rearrange("(cc p) m -> p cc m", p=P))
        alloc_qk(0)
        TAGS = ["qkv", "qkv", "st", "st", "ao", "ao"]
        waves = [
            [("k", 0), ("k", 1), ("k", 2), ("k", 3), ("q", 0), ("q", 1)],
            [("q", 2), ("q", 3), ("v", 0), ("v", 1), ("v", 2), ("v", 3)],
            [("v", 4), ("v", 5), ("v", 6), ("v", 7), ("v", 8), ("v", 9)],
            [("v", 10), ("v", 11), ("v", 12), ("v", 13), ("v", 14), ("v", 15)],
        ]
        for wi, wave in enumerate(waves):
            tiles = []
            for g, (kind, idx) in enumerate(wave):
                shape = [P, CH] if kind == "v" else [P, BLK]
                tiles.append(ps.tile(shape, F32, tag=TAGS[g], bufs=2, name=f"w{wi}g{g}"))
            for c in range(CC):
                for g, (kind, idx) in enumerate(wave):
                    if kind == "v":
                        nc.tensor.matmul(tiles[g][:], xT_c[c][:, idx * P:(idx + 1) * P],
                                         wvT_sb[:, c, :], start=(c == 0), stop=(c == CC - 1))
                    else:
                        w = wq_view(0) if kind == "q" else wk_view(0)
                        nc.tensor.matmul(tiles[g][:], w[:, c, :], xT_c[c][:, idx * BLK:(idx + 1) * BLK],
                                         start=(c == 0), stop=(c == CC - 1))
            for g, (kind, idx) in enumerate(wave):
                if kind == "v":
                    nc.vector.tensor_copy(v_sb[:, idx, :, 0:D],
                                          tiles[g][:].rearrange("p (h d) -> p h d", h=NH))
                else:
                    dst = (qT[0] if kind == "q" else kT[0])[idx]
                    bias = bq_sb if kind == "q" else bk_sb
                    nc.vector.tensor_scalar_add(dst[:], tiles[g][:], bias[:, 0:1])

        # prefetch the projection weights early: the wbig slot frees as soon as
        # the last V matmul has read wvT
        wpT_sb = sb.tile([P, NPAIR, C], BF16, tag="wbig", name="wpT_sb")
        nc.gpsimd.dma_start(wpT_sb[:], wpT.rearrange("(cp p) c -> p cp c", p=P))
        beff_sb = sb.tile([P, CC], F32, tag="beff")
        nc.scalar.dma_start(beff_sb[:], beff.rearrange("(t p) -> p t", p=P))

        def qk_group_gen(pair, which, blk):
            """Generator: one q^T/k^T block emitted one instruction at a time."""
            pps = ps.tile([P, BLK], F32, tag="qkv", bufs=2, name=f"{which}ps{pair}_{blk}")
            w = wq_view(pair) if which == "q" else wk_view(pair)
            for c in range(CC):
                mm = nc.tensor.matmul(pps[:], w[:, c, :], xT_c[c][:, blk * BLK:(blk + 1) * BLK],
                                      start=(c == 0), stop=(c == CC - 1))
                yield mm
            dst = (qT[pair] if which == "q" else kT[pair])[blk]
            bias = bq_sb if which == "q" else bk_sb
            nc.vector.tensor_scalar_add(dst[:], pps[:], bias[:, pair:pair + 1])
            yield None

        attnTs = [None] * NPAIR

        def proj_gen(blk, late=False):
            """Output projection for one n-block: yT[:, blk] = sum_cp
            wpT[cp]^T @ attnT[cp][:, blk] (+beff).  Runs as PE filler work
            inside pair-3 attention once attnT[3][:, blk] is normalized.
            The tail block (late=True) runs after all attention PSUM users
            are done, so it can rotate over more PSUM banks."""
            for ct in range(CC):
                ptag = ("qkv" if ct % 2 == 0 else "st") if late else "qkv"
                yp = ps.tile([P, BLK], F32, tag=ptag, bufs=2, name=f"yp{ct}_{blk}")
                for cp in range(NPAIR):
                    mm = nc.tensor.matmul(yp[:], wpT_sb[:, cp, ct * P:(ct + 1) * P],
                                          attnTs[cp][:, blk * BLK:(blk + 1) * BLK],
                                          start=(cp == 0), stop=(cp == NPAIR - 1))
                    yield mm
                y_sb = sb.tile([P, BLK], BF16, tag="ysb", bufs=4, name=f"ysb{ct}_{blk}")
                nc.vector.tensor_scalar_add(y_sb[:], yp[:], beff_sb[:, ct:ct + 1])
                eng = nc.sync if ct % 2 == 0 else nc.gpsimd
                eng.dma_start(yT[ct * P:(ct + 1) * P, blk * BLK:(blk + 1) * BLK], y_sb[:])
                yield None

        def normalize_blk(attnT_p, recip_d, blk):
            """attnT[:, blk] *= 1/rowsum, via a DRAM broadcast-load of the
            per-head reciprocal rows (partition-replicate) + in-place DVE mult."""
            rb = sb.tile([P, BLK], BF16, tag="rb", bufs=3, name=f"rb{blk}")
            nc.sync.dma_start(rb[0:D, :],
                              recip_d[blk:blk + 1, :].to_broadcast((D, BLK)))
            nc.gpsimd.dma_start(rb[D:2 * D, :],
                                recip_d[NBLK + blk:NBLK + blk + 1, :].to_broadcast((D, BLK)))
            sl = attnT_p[:, blk * BLK:(blk + 1) * BLK]
            nc.vector.tensor_tensor(sl, sl, rb[:], mybir.AluOpType.mult)

        def attn_pair(pair, filler, carry):
            """Attention for one head-pair; calls filler() once per inner j step
            to interleave independent PE work behind the ACT-bound exp stream.
            PV matmuls are emitted one j2-iteration late so the in-order PE
            queue never parks on an exp wait while S work is available.
            Block/pair boundary work (last PVs, copybacks, reciprocal chain,
            normalize) is deferred via `carry` into the NEXT block's first
            iteration, so the next block's S matmuls stay at the front of the
            PE queue and the exp stream never stalls at boundaries."""
            attnT_p = sb.tile([P, N], BF16, tag="attnT", bufs=NPAIR, name=f"attnT{pair}")
            attnTs[pair] = attnT_p
            recip_d = dr.tile([2 * NBLK, BLK], BF16, tag="recipd", bufs=2, name=f"recipd{pair}")
            last = pair == NPAIR - 1
            for blk in range(NBLK):
                aoA = ps.tile([D + 1, BLK], F32, tag="ao", bufs=2, name=f"aoA{pair}_{blk}")
                aoB = ps.tile([D + 1, BLK], F32, tag="ao", bufs=2, name=f"aoB{pair}_{blk}")
                pv_prev = []
                for j2 in range(NT // 2):
                    js = (2 * j2, 2 * j2 + 1)
                    pTs = {}
                    for j in js:
                        st = ps.tile([P, 2 * BLK], F32, tag="st", bufs=2, name=f"st{pair}_{blk}_{j}")
                        kt_b = kT[pair][j // 4]
                        q_b = qT[pair][blk]
                        jc = (j % 4) * P
                        nc.tensor.matmul(st[:, 0:BLK], kt_b[0:D, jc:jc + P],
                                         q_b[0:D, :],
                                         start=True, stop=True, tile_position=(0, 0))
                        nc.tensor.matmul(st[:, BLK:2 * BLK], kt_b[D:2 * D, jc:jc + P],
                                         q_b[D:2 * D, :],
                                         start=True, stop=True, tile_position=(64, 0))
                        pT = sb.tile([P, 2 * BLK], F32R, tag="pT", bufs=4, name=f"pT{pair}_{blk}_{j}")
                        nc.scalar.activation(pT[:], st[:], AF.Exp)
                        pTs[j] = pT
                    if carry:
                        for w in carry:
                            w()
                        carry.clear()
                    if pv_prev:
                        pv_prev[0]()
                        filler()
                        pv_prev[1]()
                    else:
                        filler()
                    filler(); filler()

                    def make_pv(j, pT, aoA=aoA, aoB=aoB, pair=pair):
                        def emit():
                            nc.tensor.matmul(aoA[:], v_sb[:, j, 2 * pair, :], pT[:, 0:BLK],
                                             start=(j == 0), stop=(j == NT - 1))
                            nc.tensor.matmul(aoB[:], v_sb[:, j, 2 * pair + 1, :], pT[:, BLK:2 * BLK],
                                             start=(j == 0), stop=(j == NT - 1))
                        return emit
                    pv_prev = [make_pv(j, pTs[j]) for j in js]

                def boundary(pv=pv_prev, aoA=aoA, aoB=aoB, blk=blk,
                             attnT_p=attnT_p, recip_d=recip_d, pair=pair, last=last):
                    for emit in pv:
                        emit()
                    nc.vector.tensor_copy(attnT_p[0:D, blk * BLK:(blk + 1) * BLK], aoA[0:D, :])
                    nc.vector.tensor_copy(attnT_p[D:2 * D, blk * BLK:(blk + 1) * BLK], aoB[0:D, :])
                    # per-row reciprocal on partition-0 tiles (DVE needs aligned
                    # partition bases), then stage the bf16 recip rows in DRAM
                    # for the partition-replicating broadcast loads
                    for hip, ao in ((0, aoA), (1, aoB)):
                        r = hip * NBLK + blk
                        srow = sb.tile([1, BLK], F32, tag="srow", bufs=4, name=f"srow{pair}_{r}")
                        nc.vector.tensor_copy(srow[:], ao[D:D + 1, :])
                        rrec = sb.tile([1, BLK], F32, tag="rrec", bufs=4, name=f"rrec{pair}_{r}")
                        nc.vector.reciprocal_approx_fast(rrec[:], srow[:])
                        rb16 = sb.tile([1, BLK], BF16, tag="rb16", bufs=4, name=f"rb16{pair}_{r}")
                        nc.vector.tensor_copy(rb16[:], rrec[:])
                        nc.sync.dma_start(recip_d[r:r + 1, :], rb16[:])
                    if last:
                        # per-block normalize + enqueue this block's projection
                        # as filler work for the remaining attention
                        normalize_blk(attnT_p, recip_d, blk)
                        pending.append(proj_gen(blk, late=(blk == NBLK - 1)))
                    elif blk == NBLK - 1:
                        # whole-pair normalize; overlaps the next pair's attention
                        for b in range(NBLK):
                            normalize_blk(attnT_p, recip_d, b)
                carry.append(boundary)

        # ---- attention pairs: a single GLOBAL work queue of generators feeds
        # filler() so leftover ops roll into the NEXT pair's emission instead
        # of bunching up at pair boundaries and starving ACT.
        pending = []

        def filler():
            while pending:
                try:
                    next(pending[0])
                    return
                except StopIteration:
                    pending.pop(0)

        carry = []
        for pair in range(NPAIR):
            if pair + 1 < NPAIR:
                alloc_qk(pair + 1)
                pending.extend([qk_group_gen(pair + 1, "k", blk) for blk in range(NBLK)] +
                               [qk_group_gen(pair + 1, "q", blk) for blk in range(NBLK)])
            attn_pair(pair, filler, carry)
        for w in carry:
            w()
        carry.clear()
        while pending:
            filler()

    nc.compile()
    return nc


_prog = None


def _get_program():
    global _prog
    if _prog is None:
        _prog = build_program()
    return _prog


def _prep_core_inputs(x, w_qkv, b_qkv, w_proj, b_proj, b, hg):
    scale = np.float32(D ** -0.5)
    hs = slice(hg * CH, (hg + 1) * CH)
    wq = w_qkv[0 * C:1 * C][hs]          # [CH, C]
    wk = w_qkv[1 * C:2 * C][hs]
    wv = w_qkv[2 * C:3 * C][hs]
    bqs = b_qkv[0 * C:1 * C][hs] * scale
    bks = b_qkv[1 * C:2 * C][hs]
    bvs = b_qkv[2 * C:3 * C][hs]
    wp = w_proj[:, hs]                   # [C, CH]
    beff = wp.astype(np.float64) @ bvs.astype(np.float64)
    beff = beff.astype(np.float32)
    if hg == 0:
        beff = beff + b_proj
    return {
        "xT": np.ascontiguousarray(x[b].T).astype(BF),
        "wqT": np.ascontiguousarray(wq.T * scale).astype(BF),
        "wkT": np.ascontiguousarray(wk.T).astype(BF),
        "wvT": np.ascontiguousarray(wv.T).astype(BF),
        "bq": np.ascontiguousarray(bqs),
        "bk": np.ascontiguousarray(bks),
        "wpT": np.ascontiguousarray(wp.T).astype(BF),
        "beff": np.ascontiguousarray(beff),
        "ones_in": np.ones(P, dtype=np.float32),
    }


def kernel(x, w_qkv, b_qkv, w_proj, b_proj, _trace=False, _tmpdir=None):
    x = np.asarray(x, dtype=np.float32)
    w_qkv = np.asarray(w_qkv, dtype=np.float32)
    b_qkv = np.asarray(b_qkv, dtype=np.float32)
    w_proj = np.asarray(w_proj, dtype=np.float32)
    b_proj = np.asarray(b_proj, dtype=np.float32)

    nc = _get_program()
    in_maps = [_prep_core_inputs(x, w_qkv, b_qkv, w_proj, b_proj, c // 2, c % 2)
               for c in range(8)]
    kw = {}
    if _trace:
        kw = dict(trace=True, tmpdir=_tmpdir)
    res = run_bass_kernel_spmd(nc, in_maps, core_ids=list(range(8)), **kw)
    out = np.empty((B, N, C), dtype=np.float32)
    for b in range(B):
        out[b] = (res.results[2 * b]["yT"].astype(np.float32)
                  + res.results[2 * b + 1]["yT"].astype(np.float32)).T
    if _trace:
        kernel._last_exec_ns = res.exec_time_ns
    return out


# revision 22
# speedup vs baseline: 1.0343x; 1.0343x over previous
"""Trainium2 Bass kernel for multi-head attention (B=4, N=2048, C=1024, H=16).

Sharding: 8 cores = (batch b in 0..3) x (head-group hg in 0..1, 8 heads each).
Each core computes, for its (b, hg):
  - QKV projection for its 8 heads (bf16 matmuls, fp32 PSUM, contraction C=1024)
  - attention S^T = K Q^T per head-pair (row-packed K=64 matmuls),
    exp on ACT (no max-subtraction needed: |S|max ~ 9 << 50 clamp), PV with a
    fused ones-row producing the softmax denominators for free
  - normalization fully on-chip (approx-reciprocal + SBUF->SBUF broadcast DMA)
  - output projection y_part = attnT^T @ w_projT, per n-block, interleaved
    into the last pair's attention as PE filler work
Host sums the two partial y's per batch (proj contracts over all 16 heads).

vs v1: all heavy operands bf16 (halves DMA-in/LDWEIGHTS/SBUF), attnT for all
4 pairs stays resident (no DRAM spill/reload), softmax normalize never
round-trips DRAM, PV is software-pipelined one j-step behind exp so ACT never
waits on PE, and the projection runs per n-block inside pair-3 attention.
"""
import sys, os
sys.path.insert(0, "/opt/trn_rl_repo")
import numpy as np
import ml_dtypes
from contextlib import ExitStack

import concourse.bass as bass
import concourse.bacc as bacc
import concourse.tile as tile
import concourse.mybir as mybir
from concourse.bass_utils import run_bass_kernel_spmd

B, N, C, H, D = 4, 2048, 1024, 16, 64
P = 128
NH = H // 2              # 8 heads per core
CH = NH * D              # 512: per-core channel slice
NPAIR = NH // 2          # 4 head-pairs per core
NBLK = 4                 # nq blocks of 512
BLK = N // NBLK          # 512
NT = N // P              # 16 key tiles
CC = C // P              # 8 contraction chunks
F32 = mybir.dt.float32
F32R = mybir.dt.float32r
BF16 = mybir.dt.bfloat16
AF = mybir.ActivationFunctionType
BF = ml_dtypes.bfloat16


def build_program():
    nc = bacc.Bacc(None, target_bir_lowering=False)
    xT = nc.declare_dram_parameter("xT", [C, N], BF16, isOutput=False)
    wqT = nc.declare_dram_parameter("wqT", [C, CH], BF16, isOutput=False)
    wkT = nc.declare_dram_parameter("wkT", [C, CH], BF16, isOutput=False)
    wvT = nc.declare_dram_parameter("wvT", [C, CH], BF16, isOutput=False)
    bq = nc.declare_dram_parameter("bq", [CH], F32, isOutput=False)
    bk = nc.declare_dram_parameter("bk", [CH], F32, isOutput=False)
    wpT = nc.declare_dram_parameter("wpT", [CH, C], BF16, isOutput=False)
    beff = nc.declare_dram_parameter("beff", [C], F32, isOutput=False)
    ones_in = nc.declare_dram_parameter("ones_in", [P], F32, isOutput=False)
    yT = nc.declare_dram_parameter("yT", [C, N], BF16, isOutput=True)

    with tile.TileContext(nc) as tc, ExitStack() as ctx:
        sb = ctx.enter_context(tc.tile_pool(name="sb", bufs=1))
        ps = ctx.enter_context(tc.tile_pool(name="ps", bufs=1, space="PSUM"))
        dr = ctx.enter_context(tc.tile_pool(name="dr", bufs=1, space="DRAM"))

        # ---- loads, spread over three trigger queues so transfers start in
        # parallel: xT chunks on Sync, q/k weights + biases on Scalar (ACT is
        # idle until the first exp), wvT/wpT on GpSimd.  wq/wk load whole
        # (1KB-contiguous rows, one trigger) and pairs use views into them.
        xT_c = [sb.tile([P, N], BF16, tag="xT", bufs=CC, name=f"xTc{c}") for c in range(CC)]
        wk_all = sb.tile([P, CC, CH], BF16, tag="wk")
        wq_all = sb.tile([P, CC, CH], BF16, tag="wq")
        # pair-0 weight slices first (wave 0 needs them), bulk of pairs 1-3 after
        nc.scalar.dma_start(wk_all[:, :, 0:P],
                            wkT.rearrange("(cc p) m -> p cc m", p=P)[:, :, 0:P])
        nc.scalar.dma_start(wq_all[:, :, 0:P],
                            wqT.rearrange("(cc p) m -> p cc m", p=P)[:, :, 0:P])
        for c in range(CC):
            nc.sync.dma_start(xT_c[c][:], xT[c * P:(c + 1) * P, :])
        bk_sb = sb.tile([P, NPAIR], F32, tag="biask")
        nc.scalar.dma_start(bk_sb[:], bk.rearrange("(t p) -> p t", p=P))
        bq_sb = sb.tile([P, NPAIR], F32, tag="biasq")
        nc.scalar.dma_start(bq_sb[:], bq.rearrange("(t p) -> p t", p=P))
        v_sb = sb.tile([P, NT, NH, D + 1], F32R, tag="v")
        ones_col = sb.tile([P, 1], F32, tag="onesc")
        nc.scalar.dma_start(ones_col[:], ones_in.rearrange("(p o) -> p o", o=1))
        nc.vector.tensor_copy(v_sb[:, :, :, D:D + 1], ones_col[:].to_broadcast((P, NT, NH, 1)))
        nc.scalar.dma_start(wk_all[:, :, P:NPAIR * P],
                            wkT.rearrange("(cc p) m -> p cc m", p=P)[:, :, P:NPAIR * P])
        nc.scalar.dma_start(wq_all[:, :, P:NPAIR * P],
                            wqT.rearrange("(cc p) m -> p cc m", p=P)[:, :, P:NPAIR * P])

        def wq_view(pair):
            return wq_all[:, :, pair * P:(pair + 1) * P]

        def wk_view(pair):
            return wk_all[:, :, pair * P:(pair + 1) * P]

        qT = [None] * NPAIR
        kT = [None] * NPAIR

        def alloc_qk(pair):
            # per-block tiles: S matmuls for key-tile j / query-block b then only
            # depend on the specific q/k block copybacks, not the whole pair
            qT[pair] = [sb.tile([P, BLK], BF16, tag="qT", bufs=2 * NBLK, name=f"qT{pair}_{b}")
                        for b in range(NBLK)]
            kT[pair] = [sb.tile([P, BLK], BF16, tag="kT", bufs=2 * NBLK, name=f"kT{pair}_{b}")
                        for b in range(NBLK)]

        # ---- V for all heads + q^T/k^T for pair 0: chunk-major waves over 6
        # psum slots, so the first matmuls only wait on xT chunk 0's DMA.
        wvT_sb = sb.tile([P, CC, CH], BF16, tag="wbig")
        nc.gpsimd.dma_start(wvT_sb[:], wvT.rearrange("(cc p) m -> p cc m", p=P))
        alloc_qk(0)
        TAGS = ["qkv", "qkv", "st", "st", "ao", "ao"]
        waves = [
            [("k", 0), ("k", 1), ("k", 2), ("k", 3), ("q", 0), ("q", 1)],
            [("q", 2), ("q", 3), ("v", 0), ("v", 1), ("v", 2), ("v", 3)],
            [("v", 4), ("v", 5), ("v", 6), ("v", 7), ("v", 8), ("v", 9)],
            [("v", 10), ("v", 11), ("v", 12), ("v", 13), ("v", 14), ("v", 15)],
        ]
        for wi, wave in enumerate(waves):
            tiles = []
            for g, (kind, idx) in enumerate(wave):
                shape = [P, CH] if kind == "v" else [P, BLK]
                tiles.append(ps.tile(shape, F32, tag=TAGS[g], bufs=2, name=f"w{wi}g{g}"))
            for c in range(CC):
                for g, (kind, idx) in enumerate(wave):
                    if kind == "v":
                        nc.tensor.matmul(tiles[g][:], xT_c[c][:, idx * P:(idx + 1) * P],
                                         wvT_sb[:, c, :], start=(c == 0), stop=(c == CC - 1))
                    else:
                        w = wq_view(0) if kind == "q" else wk_view(0)
                        nc.tensor.matmul(tiles[g][:], w[:, c, :], xT_c[c][:, idx * BLK:(idx + 1) * BLK],
                                         start=(c == 0), stop=(c == CC - 1))
            for g, (kind, idx) in enumerate(wave):
                if kind == "v":
                    nc.vector.tensor_copy(v_sb[:, idx, :, 0:D],
                                          tiles[g][:].rearrange("p (h d) -> p h d", h=NH))
                else:
                    dst = (qT[0] if kind == "q" else kT[0])[idx]
                    bias = bq_sb if kind == "q" else bk_sb
                    nc.vector.tensor_scalar_add(dst[:], tiles[g][:], bias[:, 0:1])

        # prefetch the projection weights early: the wbig slot frees as soon as
        # the last V matmul has read wvT
        wpT_sb = sb.tile([P, NPAIR, C], BF16, tag="wbig", name="wpT_sb")
        nc.gpsimd.dma_start(wpT_sb[:], wpT.rearrange("(cp p) c -> p cp c", p=P))
        beff_sb = sb.tile([P, CC], F32, tag="beff")
        nc.gpsimd.dma_start(beff_sb[:], beff.rearrange("(t p) -> p t", p=P))

        def qk_group_gen(pair, which, blk):
            """Generator: one q^T/k^T block emitted one instruction at a time."""
            pps = ps.tile([P, BLK], F32, tag="qkv", bufs=2, name=f"{which}ps{pair}_{blk}")
            w = wq_view(pair) if which == "q" else wk_view(pair)
            for c in range(CC):
                mm = nc.tensor.matmul(pps[:], w[:, c, :], xT_c[c][:, blk * BLK:(blk + 1) * BLK],
                                      start=(c == 0), stop=(c == CC - 1))
                yield mm
            dst = (qT[pair] if which == "q" else kT[pair])[blk]
            bias = bq_sb if which == "q" else bk_sb
            nc.vector.tensor_scalar_add(dst[:], pps[:], bias[:, pair:pair + 1])
            yield None

        attnTs = [None] * NPAIR

        def proj_gen(blk, late=False):
            """Output projection for one n-block: yT[:, blk] = sum_cp
            wpT[cp]^T @ attnT[cp][:, blk] (+beff).  Runs as PE filler work
            inside pair-3 attention once attnT[3][:, blk] is normalized.
            The tail block (late=True) runs after all attention PSUM users
            are done, so it can rotate over more PSUM banks."""
            for ct in range(CC):
                ptag = ("qkv" if ct % 2 == 0 else "st") if late else "qkv"
                yp = ps.tile([P, BLK], F32, tag=ptag, bufs=2, name=f"yp{ct}_{blk}")
                for cp in range(NPAIR):
                    mm = nc.tensor.matmul(yp[:], wpT_sb[:, cp, ct * P:(ct + 1) * P],
                                          attnTs[cp][:, blk * BLK:(blk + 1) * BLK],
                                          start=(cp == 0), stop=(cp == NPAIR - 1))
                    yield mm
                y_sb = sb.tile([P, BLK], BF16, tag="ysb", bufs=4, name=f"ysb{ct}_{blk}")
                nc.vector.tensor_scalar_add(y_sb[:], yp[:], beff_sb[:, ct:ct + 1])
                eng = nc.sync if ct % 2 == 0 else nc.gpsimd
                eng.dma_start(yT[ct * P:(ct + 1) * P, blk * BLK:(blk + 1) * BLK], y_sb[:])
                yield None

        def normalize_blk(attnT_p, recip_d, blk):
            """attnT[:, blk] *= 1/rowsum, via a DRAM broadcast-load of the
            per-head reciprocal rows (partition-replicate) + in-place DVE mult."""
            rb = sb.tile([P, BLK], BF16, tag="rb", bufs=3, name=f"rb{blk}")
            nc.sync.dma_start(rb[0:D, :],
                              recip_d[blk:blk + 1, :].to_broadcast((D, BLK)))
            nc.gpsimd.dma_start(rb[D:2 * D, :],
                                recip_d[NBLK + blk:NBLK + blk + 1, :].to_broadcast((D, BLK)))
            sl = attnT_p[:, blk * BLK:(blk + 1) * BLK]
            nc.vector.tensor_tensor(sl, sl, rb[:], mybir.AluOpType.mult)

        def attn_pair(pair, filler):
            """Attention for one head-pair; calls filler() once per inner j step
            to interleave independent PE work behind the ACT-bound exp stream.
            PV matmuls are emitted one j2-iteration late so the in-order PE
            queue never parks on an exp wait while S work is available."""
            attnT_p = sb.tile([P, N], BF16, tag="attnT", bufs=NPAIR, name=f"attnT{pair}")
            attnTs[pair] = attnT_p
            recip_d = dr.tile([2 * NBLK, BLK], BF16, tag="recipd", bufs=2, name=f"recipd{pair}")
            last = pair == NPAIR - 1
            for blk in range(NBLK):
                aoA = ps.tile([D + 1, BLK], F32, tag="ao", bufs=2, name=f"aoA{pair}_{blk}")
                aoB = ps.tile([D + 1, BLK], F32, tag="ao", bufs=2, name=f"aoB{pair}_{blk}")
                pv_prev = []
                for j2 in range(NT // 2):
                    js = (2 * j2, 2 * j2 + 1)
                    pTs = {}
                    for j in js:
                        st = ps.tile([P, 2 * BLK], F32, tag="st", bufs=2, name=f"st{pair}_{blk}_{j}")
                        kt_b = kT[pair][j // 4]
                        q_b = qT[pair][blk]
                        jc = (j % 4) * P
                        nc.tensor.matmul(st[:, 0:BLK], kt_b[0:D, jc:jc + P],
                                         q_b[0:D, :],
                                         start=True, stop=True, tile_position=(0, 0))
                        nc.tensor.matmul(st[:, BLK:2 * BLK], kt_b[D:2 * D, jc:jc + P],
                                         q_b[D:2 * D, :],
                                         start=True, stop=True, tile_position=(64, 0))
                        pT = sb.tile([P, 2 * BLK], F32R, tag="pT", bufs=4, name=f"pT{pair}_{blk}_{j}")
                        nc.scalar.activation(pT[:], st[:], AF.Exp)
                        pTs[j] = pT
                    if pv_prev:
                        pv_prev[0]()
                        filler()
                        pv_prev[1]()
                    else:
                        filler()
                    filler(); filler()

                    def make_pv(j, pT, aoA=aoA, aoB=aoB, pair=pair):
                        def emit():
                            nc.tensor.matmul(aoA[:], v_sb[:, j, 2 * pair, :], pT[:, 0:BLK],
                                             start=(j == 0), stop=(j == NT - 1))
                            nc.tensor.matmul(aoB[:], v_sb[:, j, 2 * pair + 1, :], pT[:, BLK:2 * BLK],
                                             start=(j == 0), stop=(j == NT - 1))
                        return emit
                    pv_prev = [make_pv(j, pTs[j]) for j in js]

                for emit in pv_prev:
                    emit()
                nc.vector.tensor_copy(attnT_p[0:D, blk * BLK:(blk + 1) * BLK], aoA[0:D, :])
                nc.vector.tensor_copy(attnT_p[D:2 * D, blk * BLK:(blk + 1) * BLK], aoB[0:D, :])
                # per-row reciprocal on partition-0 tiles (DVE needs aligned
                # partition bases), then stage the bf16 recip rows in DRAM for
                # the partition-replicating broadcast loads
                for hip, ao in ((0, aoA), (1, aoB)):
                    r = hip * NBLK + blk
                    srow = sb.tile([1, BLK], F32, tag="srow", bufs=4, name=f"srow{pair}_{r}")
                    nc.vector.tensor_copy(srow[:], ao[D:D + 1, :])
                    rrec = sb.tile([1, BLK], F32, tag="rrec", bufs=4, name=f"rrec{pair}_{r}")
                    nc.vector.reciprocal_approx_fast(rrec[:], srow[:])
                    rb16 = sb.tile([1, BLK], BF16, tag="rb16", bufs=4, name=f"rb16{pair}_{r}")
                    nc.vector.tensor_copy(rb16[:], rrec[:])
                    nc.sync.dma_start(recip_d[r:r + 1, :], rb16[:])
                if last:
                    # per-block normalize + enqueue this block's projection
                    # as filler work for the remaining attention
                    normalize_blk(attnT_p, recip_d, blk)
                    pending.append(proj_gen(blk, late=(blk == NBLK - 1)))
            if not last:
                # whole-pair normalize; overlaps the next pair's attention
                for b in range(NBLK):
                    normalize_blk(attnT_p, recip_d, b)

        # ---- attention pairs: a single GLOBAL work queue of generators feeds
        # filler() so leftover ops roll into the NEXT pair's emission instead
        # of bunching up at pair boundaries and starving ACT.
        pending = []

        def filler():
            while pending:
                try:
                    next(pending[0])
                    return
                except StopIteration:
                    pending.pop(0)

        for pair in range(NPAIR):
            if pair + 1 < NPAIR:
                alloc_qk(pair + 1)
                pending.extend([qk_group_gen(pair + 1, "k", blk) for blk in range(NBLK)] +
                               [qk_group_gen(pair + 1, "q", blk) for blk in range(NBLK)])
            attn_pair(pair, filler)
        while pending:
            filler()

    nc.compile()
    return nc


_prog = None


def _get_program():
    global _prog
    if _prog is None:
        _prog = build_program()
    return _prog


def _prep_core_inputs(x, w_qkv, b_qkv, w_proj, b_proj, b, hg):
    scale = np.float32(D ** -0.5)
    hs = slice(hg * CH, (hg + 1) * CH)
    wq = w_qkv[0 * C:1 * C][hs]          # [CH, C]
    wk = w_qkv[1 * C:2 * C][hs]
    wv = w_qkv[2 * C:3 * C][hs]
    bqs = b_qkv[0 * C:1 * C][hs] * scale
    bks = b_qkv[1 * C:2 * C][hs]
    bvs = b_qkv[2 * C:3 * C][hs]
    wp = w_proj[:, hs]                   # [C, CH]
    beff = wp.astype(np.float64) @ bvs.astype(np.float64)
    beff = beff.astype(np.float32)
    if hg == 0:
        beff = beff + b_proj
    return {
        "xT": np.ascontiguousarray(x[b].T).astype(BF),
        "wqT": np.ascontiguousarray(wq.T * scale).astype(BF),
        "wkT": np.ascontiguousarray(wk.T).astype(BF),
        "wvT": np.ascontiguousarray(wv.T).astype(BF),
        "bq": np.ascontiguousarray(bqs),
        "bk": np.ascontiguousarray(bks),
        "wpT": np.ascontiguousarray(wp.T).astype(BF),
        "beff": np.ascontiguousarray(beff),
        "ones_in": np.ones(P, dtype=np.float32),
    }


def kernel(x, w_qkv, b_qkv, w_proj, b_proj, _trace=False, _tmpdir=None):
    x = np.asarray(x, dtype=np.float32)
    w_qkv = np.asarray(w_qkv, dtype=np.float32)
    b_qkv = np.asarray(b_qkv, dtype=np.float32)
    w_proj = np.asarray(w_proj, dtype=np.float32)
    b_proj = np.asarray(b_proj, dtype=np.float32)

    nc = _get_program()
    in_maps = [_prep_core_inputs(x, w_qkv, b_qkv, w_proj, b_proj, c // 2, c % 2)
               for c in range(8)]
    kw = {}
    if _trace:
        kw = dict(trace=True, tmpdir=_tmpdir)
    res = run_bass_kernel_spmd(nc, in_maps, core_ids=list(range(8)), **kw)
    out = np.empty((B, N, C), dtype=np.float32)
    for b in range(B):
        out[b] = (res.results[2 * b]["yT"].astype(np.float32)
                  + res.results[2 * b + 1]["yT"].astype(np.float32)).T
    if _trace:
        kernel._last_exec_ns = res.exec_time_ns
    return out
